# revision 31
# baseline (speedup 1.0000x reference)
"""TRN2 Bass kernel for additive-attention nn.Module (B=8, X=Y=2048, EMB=DEC=1024).

Sharding: pure data-parallel, one batch element per NeuronCore (8 cores).

Per-core math (b fixed):
  q  = (state @ W_in + b_in + prev) / sqrt(2)        [Y, E]
  a  = q @ ctx^T  (+ -inf mask over x)               [Y, X]
  P  = exp(a - C)*mask      (C fixed shift; softmax is shift-invariant)
  sig[y] = sum_x P[x, y]
  c  = P-weighted sum of ctx_plus_emb                [Y, E]
  out = (c * sqrt(len)/sig) @ W_out + b_out          [Y, D]

Device layouts are transposed so every matmul operand is natural:
  qT[e,y] (phase A) -> alphaT[x,y] -> PT[x,y] (B1) -> cT[e,y] -> out[y,d] (B2).
Host pre-transposes state/prev/ctx (a layout choice of the shard), folds b_in
and the 1/sqrt(2) into prevT/W_in, and converts the value path to bf16.
Score path runs in float32r (full PE rate, ~1.5e-4 matmul rel-err).
"""
import math

import numpy as np
import ml_dtypes

import concourse.tile as tile
from concourse import bacc, mybir
from concourse.bass_utils import run_bass_kernel_spmd

B, X, Y, E, D = 8, 2048, 2048, 1024, 1024
C_SHIFT = 135.0
NEG_BIG = -1.0e30

F32 = mybir.dt.float32
F32R = mybir.dt.float32r
BF16 = mybir.dt.bfloat16

XT, YT, ET, DT = X // 128, Y // 128, E // 128, D // 128  # 16, 16, 8, 8
NC = 4            # y chunks in phase B
CS = Y // NC      # 512
SUBS = CS // 128  # 4 y subtiles per chunk
ANC = 8           # y blocks in phase A
ACS = Y // ANC    # 256


def build_nc(repeat=1):
    nc = bacc.Bacc("TRN2", target_bir_lowering=False, debug=False)
    stateT = nc.declare_dram_parameter("stateT", [D, Y], F32R, isOutput=False)
    prevT = nc.declare_dram_parameter("prevT", [E, Y], F32, isOutput=False)
    Win = nc.declare_dram_parameter("Win", [D, E], F32R, isOutput=False)
    ctxT = nc.declare_dram_parameter("ctxT", [E, X], F32R, isOutput=False)
    cp = nc.declare_dram_parameter("cp", [X, E], BF16, isOutput=False)
    Wout = nc.declare_dram_parameter("Wout", [E, D], BF16, isOutput=False)
    mbias = nc.declare_dram_parameter("mbias", [128, XT], F32, isOutput=False)
    sl = nc.declare_dram_parameter("sl", [128, 1], F32, isOutput=False)
    bout = nc.declare_dram_parameter("bout", [128, D], BF16, isOutput=False)
    out_dram = nc.declare_dram_parameter("out", [Y, D], F32, isOutput=True)

    with tile.TileContext(nc) as tc:
        with tc.tile_pool(name="glob", bufs=1) as glob:

            def body():
                # ---- small constants ----
                mb_sb = glob.tile([128, XT], F32, tag="mb", name="mb")
                nc.sync.dma_start(out=mb_sb, in_=mbias[:])
                sl_sb = glob.tile([128, 1], F32, tag="sl", name="sl")
                nc.sync.dma_start(out=sl_sb, in_=sl[:])
                ones_bf = glob.tile([128, 1], BF16, tag="ones", name="ones")
                nc.vector.memset(ones_bf, 1.0)
                one11 = glob.tile([1, 1], F32, tag="one11", name="one11")
                nc.vector.memset(one11, 1.0)
                r2_sb = glob.tile([128, YT], F32, tag="r2", name="r2")

                # resident ctxT tiles split into x-quarters; DMAs issued after
                # phase A's stream loads (in quarter order) so each B1 x-group
                # starts as soon as its quarter has landed
                ctx_tq = [
                    [
                        glob.tile(
                            [128, X // 4], F32R,
                            tag=f"ctxT{e}q{q}", name=f"ctxT{e}q{q}",
                        )
                        for q in range(4)
                    ]
                    for e in range(ET)
                ]

                # qt[c]/pt[c] timeshare 16KB slots: tag qp{c} holds qt_c then pt_{c+1}
                def qt_tile(c):
                    return glob.tile(
                        [128, ET, CS], F32R, tag=f"qp{c}", name=f"qt{c}"
                    )

                def pt_tile(c):
                    tg = f"qp{c - 1}" if c > 0 else f"qp{NC}"
                    return glob.tile([128, XT, CS], BF16, tag=tg, name=f"pt{c}")

                # ---- phase A: qT = Win.T @ stateT + prevT ----
                qt = []
                with (
                    tc.tile_pool(name="pa", bufs=1) as pa,
                    tc.tile_pool(name="psA", bufs=3, space="PSUM") as psA,
                ):
                    # ~100 dependency-free matmuls on the memset ones tile run
                    # during the initial DMA ramp and un-throttle the PE clock
                    # (HAM) before the first real matmul arrives
                    warm = psA.tile([1, 1], F32, tag="warm", name="warm")
                    for w in range(100):
                        nc.tensor.matmul(
                            warm, ones_bf, ones_bf, start=(w == 0), stop=(w == 99)
                        )

                    win_sb = pa.tile([128, DT, E], F32R, tag="win", name="win")
                    nc.sync.dma_start(
                        out=win_sb, in_=Win.rearrange("(t p) e -> p t e", p=128)
                    )
                    for ab in range(ANC):
                        st = pa.tile([128, DT, ACS], F32R, tag="st", bufs=2, name="st")
                        nc.sync.dma_start(
                            out=st,
                            in_=stateT[:, ab * ACS : (ab + 1) * ACS].rearrange(
                                "(t p) y -> p t y", p=128
                            ),
                        )
                        pv = pa.tile([128, ET, ACS], F32, tag="pv", bufs=1, name="pv")
                        nc.sync.dma_start(
                            out=pv,
                            in_=prevT[:, ab * ACS : (ab + 1) * ACS].rearrange(
                                "(t p) y -> p t y", p=128
                            ),
                        )
                        if ab % 2 == 0:
                            qt.append(qt_tile(ab // 2))
                        q = qt[ab // 2]
                        off = (ab % 2) * ACS
                        for e in range(ET):
                            ps = psA.tile([128, ACS], F32, tag="psA", name="psA")
                            for d in range(DT):
                                nc.tensor.matmul(
                                    ps,
                                    win_sb[:, d, e * 128 : (e + 1) * 128],
                                    st[:, d, :],
                                    start=(d == 0),
                                    stop=(d == DT - 1),
                                )
                            nc.vector.tensor_add(
                                q[:, e, off : off + ACS], ps, pv[:, e, :]
                            )

                # ctxT DMAs issued after all st/pv so the A-tail is not
                # starved; quarters land in x order and stream under B1
                for q in range(4):
                    for e in range(ET):
                        nc.sync.dma_start(
                            out=ctx_tq[e][q],
                            in_=ctxT[
                                e * 128 : (e + 1) * 128,
                                q * (X // 4) : (q + 1) * (X // 4),
                            ],
                        )

                # ---- phase B: B1 scores/exp/sigma + B2 weighted sum/out ----
                with (
                    tc.tile_pool(name="pb", bufs=1) as pb,
                    tc.tile_pool(name="psB", bufs=3, space="PSUM") as psB,
                    tc.tile_pool(name="psSig", bufs=1, space="PSUM") as psSig,
                    tc.tile_pool(name="psC", bufs=2, space="PSUM") as psC,
                    tc.tile_pool(name="psO", bufs=2, space="PSUM") as psO,
                ):
                    cp_sb = pb.tile([128, XT, E], BF16, tag="cp", name="cp")
                    nc.sync.dma_start(
                        out=cp_sb, in_=cp.rearrange("(t p) e -> p t e", p=128)
                    )
                    wout_sb = pb.tile([128, ET, D], BF16, tag="wout", name="wout")
                    nc.sync.dma_start(
                        out=wout_sb, in_=Wout.rearrange("(t p) d -> p t d", p=128)
                    )
                    bout_sb = pb.tile([128, D], BF16, tag="bout", name="bout")
                    nc.sync.dma_start(out=bout_sb, in_=bout[:])

                    for c in range(NC):
                        # B1: scores + exp
                        p = pt_tile(c)
                        for x in range(XT):
                            aps = psB.tile([128, CS], F32, tag="psB", name="psB")
                            for e in range(ET):
                                nc.tensor.matmul(
                                    aps,
                                    ctx_tq[e][x // 4][:, (x % 4) * 128 : (x % 4 + 1) * 128],
                                    qt[c][:, e, :],
                                    start=(e == 0),
                                    stop=(e == ET - 1),
                                )
                            nc.scalar.activation(
                                p[:, x, :],
                                aps,
                                mybir.ActivationFunctionType.Exp,
                                bias=mb_sb[:, x : x + 1],
                            )
                        # sigma: ones-row matmul -> [1,CS], then K=1 matmuls
                        # column-ize each 128-slice back onto partitions
                        sig_row = psSig.tile(
                            [1, CS], F32, tag="sigrow", name="sig_row"
                        )
                        for x in range(XT):
                            nc.tensor.matmul(
                                sig_row,
                                ones_bf,
                                p[:, x, :],
                                start=(x == 0),
                                stop=(x == XT - 1),
                            )
                        srow_sb = pb.tile([1, CS], F32, tag="osb", name="srow_sb")
                        nc.vector.tensor_copy(srow_sb, sig_row)

                        # B2: cT then out projection; the sigma transpose/recip
                        # chain is emitted after mm3's first e-group so the PE
                        # never waits on the srow DVE copy
                        ct = pb.tile([128, ET, CS], BF16, tag="ct", bufs=1, name="ct")
                        rc = None
                        for e in range(ET):
                            cps = psC.tile([128, CS], F32, tag="cps", name="cps")
                            for x in range(XT):
                                nc.tensor.matmul(
                                    cps,
                                    cp_sb[:, x, e * 128 : (e + 1) * 128],
                                    p[:, x, :],
                                    start=(x == 0),
                                    stop=(x == XT - 1),
                                )
                            nc.vector.tensor_copy(ct[:, e, :], cps)
                            if e == 0:
                                rc = pb.tile(
                                    [128, SUBS], F32, tag="rc", bufs=2, name="rc"
                                )
                                for s in range(SUBS):
                                    tps = psSig.tile(
                                        [128, 1], F32, tag="sigrow", name="tps"
                                    )
                                    nc.tensor.matmul(
                                        tps,
                                        srow_sb[0:1, s * 128 : (s + 1) * 128],
                                        one11,
                                    )
                                    nc.vector.reciprocal(rc[:, s : s + 1], tps)
                                cols = slice(c * SUBS, (c + 1) * SUBS)
                                nc.vector.tensor_scalar_mul(
                                    r2_sb[:, cols], rc, sl_sb
                                )
                        for s in range(SUBS):
                            t = c * SUBS + s
                            osb = pb.tile([128, D], F32, tag="osb", bufs=1, name="osb")
                            for ch in range(2):
                                ops = psO.tile([128, 512], F32, tag="ops", name="ops")
                                for e in range(ET):
                                    nc.tensor.matmul(
                                        ops,
                                        ct[:, e, s * 128 : (s + 1) * 128],
                                        wout_sb[:, e, ch * 512 : (ch + 1) * 512],
                                        start=(e == 0),
                                        stop=(e == ET - 1),
                                    )
                                nc.scalar.activation(
                                    osb[:, ch * 512 : (ch + 1) * 512],
                                    ops,
                                    mybir.ActivationFunctionType.Copy,
                                    scale=r2_sb[:, t : t + 1],
                                )
                                nc.vector.tensor_add(
                                    osb[:, ch * 512 : (ch + 1) * 512],
                                    osb[:, ch * 512 : (ch + 1) * 512],
                                    bout_sb[:, ch * 512 : (ch + 1) * 512],
                                )
                            nc.sync.dma_start(
                                out=out_dram[t * 128 : (t + 1) * 128, :], in_=osb
                            )

            if repeat == 1:
                body()
            else:
                with tc.For_i(0, repeat, 1):
                    body()
    nc.compile()
    return nc


_CACHE = {}


def _get_nc():
    if "nc" not in _CACHE:
        _CACHE["nc"] = build_nc()
    return _CACHE["nc"]


def make_in_maps(ctx, ctx_plus_emb, x_mask, prev_w_emb, state_pre_attn,
                 W_in, b_in, W_out, b_out):
    s2 = 1.0 / math.sqrt(2.0)
    win = np.ascontiguousarray(np.asarray(W_in) * s2, dtype=np.float32)
    wout_bf = np.ascontiguousarray(np.asarray(W_out)).astype(ml_dtypes.bfloat16)
    bout_bc = np.ascontiguousarray(
        np.broadcast_to(
            np.asarray(b_out, dtype=np.float32).astype(ml_dtypes.bfloat16), (128, D)
        )
    )
    in_maps = []
    for b in range(B):
        statet = np.ascontiguousarray(
            np.asarray(state_pre_attn[b]).T, dtype=np.float32
        )
        prevt = np.ascontiguousarray(
            ((np.asarray(prev_w_emb[b]) + np.asarray(b_in)) * s2).T,
            dtype=np.float32,
        )
        ctxt = np.ascontiguousarray(np.asarray(ctx[b]).T, dtype=np.float32)
        cp_bf = np.ascontiguousarray(np.asarray(ctx_plus_emb[b])).astype(
            ml_dtypes.bfloat16
        )
        mask = np.asarray(x_mask[b], dtype=np.float32)
        mbias = np.where(mask == 1.0, -C_SHIFT, NEG_BIG).astype(np.float32)
        mbias = np.ascontiguousarray(mbias.reshape(XT, 128).T)
        slv = np.full((128, 1), math.sqrt(float(mask.sum())), dtype=np.float32)
        in_maps.append(
            {
                "stateT": statet,
                "prevT": prevt,
                "Win": win,
                "ctxT": ctxt,
                "cp": cp_bf,
                "Wout": wout_bf,
                "mbias": mbias,
                "sl": slv,
                "bout": bout_bc,
            }
        )
    return in_maps


def kernel(ctx, ctx_plus_emb, x_mask, prev_w_emb, state_pre_attn,
           W_in, b_in, W_out, b_out):
    nc = _get_nc()
    in_maps = make_in_maps(
        ctx, ctx_plus_emb, x_mask, prev_w_emb, state_pre_attn,
        W_in, b_in, W_out, b_out,
    )
    res = run_bass_kernel_spmd(nc, in_maps, core_ids=list(range(B)))
    return np.stack([res.results[b]["out"] for b in range(B)], axis=0)


# revision 32
# speedup vs baseline: 1.0987x; 1.0987x over previous
"""TRN2 Bass kernel for additive-attention nn.Module (B=8, X=Y=2048, EMB=DEC=1024).

Sharding: pure data-parallel, one batch element per NeuronCore (8 cores).

Per-core math (b fixed):
  q  = (state @ W_in + b_in + prev) / sqrt(2)        [Y, E]
  a  = q @ ctx^T  (+ -inf mask over x)               [Y, X]
  P  = exp(a - C)*mask      (C fixed shift; softmax is shift-invariant)
  sig[y] = sum_x P[x, y]
  c  = P-weighted sum of ctx_plus_emb                [Y, E]
  out = (c * sqrt(len)/sig) @ W_out + b_out          [Y, D]

Device layouts are transposed so every matmul operand is natural:
  qT[e,y] (phase A) -> alphaT[x,y] -> PT[x,y] (B1) -> cT[e,y] -> out[y,d] (B2).
Host pre-transposes state/prev/ctx (a layout choice of the shard), folds b_in
and the 1/sqrt(2) into prevT/W_in, and converts the value path to bf16.
Score path runs in float32r (full PE rate, ~1.5e-4 matmul rel-err).
"""
import math

import numpy as np
import ml_dtypes

import concourse.tile as tile
from concourse import bacc, mybir
from concourse.bass_utils import run_bass_kernel_spmd

B, X, Y, E, D = 8, 2048, 2048, 1024, 1024
C_SHIFT = 135.0
NEG_BIG = -1.0e30

F32 = mybir.dt.float32
F32R = mybir.dt.float32r
BF16 = mybir.dt.bfloat16

XT, YT, ET, DT = X // 128, Y // 128, E // 128, D // 128  # 16, 16, 8, 8
NC = 4            # y chunks in phase B
CS = Y // NC      # 512
SUBS = CS // 128  # 4 y subtiles per chunk
ANC = 8           # y blocks in phase A
ACS = Y // ANC    # 256


def build_nc(repeat=1):
    nc = bacc.Bacc("TRN2", target_bir_lowering=False, debug=False)
    stateT = nc.declare_dram_parameter("stateT", [D, Y], F32R, isOutput=False)
    prevT = nc.declare_dram_parameter("prevT", [E, Y], F32, isOutput=False)
    Win = nc.declare_dram_parameter("Win", [D, E], F32R, isOutput=False)
    ctxT = nc.declare_dram_parameter("ctxT", [E, X], F32R, isOutput=False)
    cp = nc.declare_dram_parameter("cp", [X, E], BF16, isOutput=False)
    Wout = nc.declare_dram_parameter("Wout", [E, D], BF16, isOutput=False)
    mbias = nc.declare_dram_parameter("mbias", [128, XT], F32, isOutput=False)
    sl = nc.declare_dram_parameter("sl", [128, 1], F32, isOutput=False)
    bout = nc.declare_dram_parameter("bout", [128, D], BF16, isOutput=False)
    out_dram = nc.declare_dram_parameter("out", [Y, D], F32, isOutput=True)

    with tile.TileContext(nc) as tc:
        with tc.tile_pool(name="glob", bufs=1) as glob:

            def body():
                # ---- small constants ----
                mb_sb = glob.tile([128, XT], F32, tag="mb", name="mb")
                nc.sync.dma_start(out=mb_sb, in_=mbias[:])
                sl_sb = glob.tile([128, 1], F32, tag="sl", name="sl")
                nc.sync.dma_start(out=sl_sb, in_=sl[:])
                ones_bf = glob.tile([128, 1], BF16, tag="ones", name="ones")
                nc.vector.memset(ones_bf, 1.0)
                one11 = glob.tile([1, 1], F32, tag="one11", name="one11")
                nc.vector.memset(one11, 1.0)
                r2_sb = glob.tile([128, YT], F32, tag="r2", name="r2")

                # resident ctxT tiles split into x-quarters; DMAs issued after
                # phase A's stream loads (in quarter order) so each B1 x-group
                # starts as soon as its quarter has landed
                ctx_tq = [
                    [
                        glob.tile(
                            [128, X // 4], F32R,
                            tag=f"ctxT{e}q{q}", name=f"ctxT{e}q{q}",
                        )
                        for q in range(4)
                    ]
                    for e in range(ET)
                ]

                # qt[c]/pt[c] timeshare 16KB slots: tag qp{c} holds qt_c then pt_{c+1}
                def qt_tile(c):
                    return glob.tile(
                        [128, ET, CS], F32R, tag=f"qp{c}", name=f"qt{c}"
                    )

                def pt_tile(c):
                    tg = f"qp{c - 1}" if c > 0 else f"qp{NC}"
                    return glob.tile([128, XT, CS], BF16, tag=tg, name=f"pt{c}")

                # ---- phase A: qT = Win.T @ stateT + prevT ----
                qt = []
                with (
                    tc.tile_pool(name="pa", bufs=1) as pa,
                    tc.tile_pool(name="psA", bufs=3, space="PSUM") as psA,
                ):
                    # ~100 dependency-free matmuls on the memset ones tile run
                    # during the initial DMA ramp and un-throttle the PE clock
                    # (HAM) before the first real matmul arrives
                    warm = psA.tile([1, 1], F32, tag="warm", name="warm")
                    for w in range(100):
                        nc.tensor.matmul(
                            warm, ones_bf, ones_bf, start=(w == 0), stop=(w == 99)
                        )

                    # win split in halves: win1 shares pt0's 16KB slot (it
                    # dies at end of A exactly when pt0 is born), freeing room
                    # for deeper st/pv stream buffering
                    win0 = pa.tile([128, DT // 2, E], F32R, tag="win", name="win0")
                    nc.sync.dma_start(
                        out=win0,
                        in_=Win[: D // 2].rearrange("(t p) e -> p t e", p=128),
                    )
                    win1 = glob.tile(
                        [128, DT // 2, E], F32R, tag=f"qp{NC}", name="win1"
                    )
                    nc.sync.dma_start(
                        out=win1,
                        in_=Win[D // 2 :].rearrange("(t p) e -> p t e", p=128),
                    )
                    for ab in range(ANC):
                        st = pa.tile([128, DT, ACS], F32R, tag="st", bufs=3, name="st")
                        nc.sync.dma_start(
                            out=st,
                            in_=stateT[:, ab * ACS : (ab + 1) * ACS].rearrange(
                                "(t p) y -> p t y", p=128
                            ),
                        )
                        pv = pa.tile([128, ET, ACS], F32, tag="pv", bufs=2, name="pv")
                        nc.sync.dma_start(
                            out=pv,
                            in_=prevT[:, ab * ACS : (ab + 1) * ACS].rearrange(
                                "(t p) y -> p t y", p=128
                            ),
                        )
                        if ab % 2 == 0:
                            qt.append(qt_tile(ab // 2))
                        q = qt[ab // 2]
                        off = (ab % 2) * ACS
                        for e in range(ET):
                            ps = psA.tile([128, ACS], F32, tag="psA", name="psA")
                            for d in range(DT):
                                wh = win0 if d < DT // 2 else win1
                                nc.tensor.matmul(
                                    ps,
                                    wh[:, d % (DT // 2), e * 128 : (e + 1) * 128],
                                    st[:, d, :],
                                    start=(d == 0),
                                    stop=(d == DT - 1),
                                )
                            nc.vector.tensor_add(
                                q[:, e, off : off + ACS], ps, pv[:, e, :]
                            )

                # ctxT DMAs issued after all st/pv so the A-tail is not
                # starved; quarters land in x order and stream under B1
                for q in range(4):
                    for e in range(ET):
                        nc.sync.dma_start(
                            out=ctx_tq[e][q],
                            in_=ctxT[
                                e * 128 : (e + 1) * 128,
                                q * (X // 4) : (q + 1) * (X // 4),
                            ],
                        )

                # ---- phase B: B1 scores/exp/sigma + B2 weighted sum/out ----
                with (
                    tc.tile_pool(name="pb", bufs=1) as pb,
                    tc.tile_pool(name="psB", bufs=3, space="PSUM") as psB,
                    tc.tile_pool(name="psSig", bufs=1, space="PSUM") as psSig,
                    tc.tile_pool(name="psC", bufs=2, space="PSUM") as psC,
                    tc.tile_pool(name="psO", bufs=2, space="PSUM") as psO,
                ):
                    cp_sb = pb.tile([128, XT, E], BF16, tag="cp", name="cp")
                    nc.sync.dma_start(
                        out=cp_sb, in_=cp.rearrange("(t p) e -> p t e", p=128)
                    )
                    wout_sb = pb.tile([128, ET, D], BF16, tag="wout", name="wout")
                    nc.sync.dma_start(
                        out=wout_sb, in_=Wout.rearrange("(t p) d -> p t d", p=128)
                    )
                    bout_sb = pb.tile([128, D], BF16, tag="bout", name="bout")
                    nc.sync.dma_start(out=bout_sb, in_=bout[:])

                    for c in range(NC):
                        # B1: scores + exp
                        p = pt_tile(c)
                        for x in range(XT):
                            aps = psB.tile([128, CS], F32, tag="psB", name="psB")
                            for e in range(ET):
                                nc.tensor.matmul(
                                    aps,
                                    ctx_tq[e][x // 4][:, (x % 4) * 128 : (x % 4 + 1) * 128],
                                    qt[c][:, e, :],
                                    start=(e == 0),
                                    stop=(e == ET - 1),
                                )
                            nc.scalar.activation(
                                p[:, x, :],
                                aps,
                                mybir.ActivationFunctionType.Exp,
                                bias=mb_sb[:, x : x + 1],
                            )
                        # sigma: ones-row matmul -> [1,CS], then K=1 matmuls
                        # column-ize each 128-slice back onto partitions
                        sig_row = psSig.tile(
                            [1, CS], F32, tag="sigrow", name="sig_row"
                        )
                        for x in range(XT):
                            nc.tensor.matmul(
                                sig_row,
                                ones_bf,
                                p[:, x, :],
                                start=(x == 0),
                                stop=(x == XT - 1),
                            )
                        srow_sb = pb.tile([1, CS], F32, tag="osb", name="srow_sb")
                        nc.vector.tensor_copy(srow_sb, sig_row)

                        # B2: cT then out projection; the sigma transpose/recip
                        # chain is emitted after mm3's first e-group so the PE
                        # never waits on the srow DVE copy
                        ct = pb.tile([128, ET, CS], BF16, tag="ct", bufs=1, name="ct")
                        rc = None
                        for e in range(ET):
                            cps = psC.tile([128, CS], F32, tag="cps", name="cps")
                            for x in range(XT):
                                nc.tensor.matmul(
                                    cps,
                                    cp_sb[:, x, e * 128 : (e + 1) * 128],
                                    p[:, x, :],
                                    start=(x == 0),
                                    stop=(x == XT - 1),
                                )
                            nc.vector.tensor_copy(ct[:, e, :], cps)
                            if e == 0:
                                rc = pb.tile(
                                    [128, SUBS], F32, tag="rc", bufs=2, name="rc"
                                )
                                for s in range(SUBS):
                                    tps = psSig.tile(
                                        [128, 1], F32, tag="sigrow", name="tps"
                                    )
                                    nc.tensor.matmul(
                                        tps,
                                        srow_sb[0:1, s * 128 : (s + 1) * 128],
                                        one11,
                                    )
                                    nc.vector.reciprocal(rc[:, s : s + 1], tps)
                                cols = slice(c * SUBS, (c + 1) * SUBS)
                                nc.vector.tensor_scalar_mul(
                                    r2_sb[:, cols], rc, sl_sb
                                )
                        for s in range(SUBS):
                            t = c * SUBS + s
                            osb = pb.tile([128, D], F32, tag="osb", bufs=1, name="osb")
                            for ch in range(2):
                                ops = psO.tile([128, 512], F32, tag="ops", name="ops")
                                for e in range(ET):
                                    nc.tensor.matmul(
                                        ops,
                                        ct[:, e, s * 128 : (s + 1) * 128],
                                        wout_sb[:, e, ch * 512 : (ch + 1) * 512],
                                        start=(e == 0),
                                        stop=(e == ET - 1),
                                    )
                                nc.scalar.activation(
                                    osb[:, ch * 512 : (ch + 1) * 512],
                                    ops,
                                    mybir.ActivationFunctionType.Copy,
                                    scale=r2_sb[:, t : t + 1],
                                )
                                nc.vector.tensor_add(
                                    osb[:, ch * 512 : (ch + 1) * 512],
                                    osb[:, ch * 512 : (ch + 1) * 512],
                                    bout_sb[:, ch * 512 : (ch + 1) * 512],
                                )
                            nc.sync.dma_start(
                                out=out_dram[t * 128 : (t + 1) * 128, :], in_=osb
                            )

            if repeat == 1:
                body()
            else:
                with tc.For_i(0, repeat, 1):
                    body()
    nc.compile()
    return nc


_CACHE = {}


def _get_nc():
    if "nc" not in _CACHE:
        _CACHE["nc"] = build_nc()
    return _CACHE["nc"]


def make_in_maps(ctx, ctx_plus_emb, x_mask, prev_w_emb, state_pre_attn,
                 W_in, b_in, W_out, b_out):
    s2 = 1.0 / math.sqrt(2.0)
    win = np.ascontiguousarray(np.asarray(W_in) * s2, dtype=np.float32)
    wout_bf = np.ascontiguousarray(np.asarray(W_out)).astype(ml_dtypes.bfloat16)
    bout_bc = np.ascontiguousarray(
        np.broadcast_to(
            np.asarray(b_out, dtype=np.float32).astype(ml_dtypes.bfloat16), (128, D)
        )
    )
    in_maps = []
    for b in range(B):
        statet = np.ascontiguousarray(
            np.asarray(state_pre_attn[b]).T, dtype=np.float32
        )
        prevt = np.ascontiguousarray(
            ((np.asarray(prev_w_emb[b]) + np.asarray(b_in)) * s2).T,
            dtype=np.float32,
        )
        ctxt = np.ascontiguousarray(np.asarray(ctx[b]).T, dtype=np.float32)
        cp_bf = np.ascontiguousarray(np.asarray(ctx_plus_emb[b])).astype(
            ml_dtypes.bfloat16
        )
        mask = np.asarray(x_mask[b], dtype=np.float32)
        mbias = np.where(mask == 1.0, -C_SHIFT, NEG_BIG).astype(np.float32)
        mbias = np.ascontiguousarray(mbias.reshape(XT, 128).T)
        slv = np.full((128, 1), math.sqrt(float(mask.sum())), dtype=np.float32)
        in_maps.append(
            {
                "stateT": statet,
                "prevT": prevt,
                "Win": win,
                "ctxT": ctxt,
                "cp": cp_bf,
                "Wout": wout_bf,
                "mbias": mbias,
                "sl": slv,
                "bout": bout_bc,
            }
        )
    return in_maps


def kernel(ctx, ctx_plus_emb, x_mask, prev_w_emb, state_pre_attn,
           W_in, b_in, W_out, b_out):
    nc = _get_nc()
    in_maps = make_in_maps(
        ctx, ctx_plus_emb, x_mask, prev_w_emb, state_pre_attn,
        W_in, b_in, W_out, b_out,
    )
    res = run_bass_kernel_spmd(nc, in_maps, core_ids=list(range(B)))
    return np.stack([res.results[b]["out"] for b in range(B)], axis=0)
